# revision 8
# baseline (speedup 1.0000x reference)
"""Trainium2 Bass kernel for nn_EvoLayer (GNN message passing layer).

Strategy (8 NeuronCores, edge-parallel by dst-range):
  - Host: sort edges by dst, shard into 8 contiguous dst node-ranges
    (1250 nodes each). Each core owns a disjoint node range, so the
    segment-sum needs no collectives.
  - Device (per core): indirect-DMA gather h[src]/h[dst] rows, PE-transpose
    to feature-major, weights-stationary MLP1 (+gelu, bias fused on ACT),
    activation-stationary MLP2, segment-sum via one-hot A-matrix matmuls
    accumulated in 128-node PSUM windows, LayerNorm on DVE, residuals.
  - Host: concat h_new shards, inverse-permute e_new.

All matmuls run as float32r (FP22 truncation, full PE rate at N>=256).
"""

import math

import numpy as np

import orjson

import concourse.bass as bass
import concourse.tile as tile
from concourse import mybir
from concourse.bass import IndirectOffsetOnAxis

F32 = mybir.dt.float32
F32R = mybir.dt.float32r
I32 = mybir.dt.int32
AF = mybir.ActivationFunctionType
ALU = mybir.AluOpType
AX = mybir.AxisListType
GELU = AF.Gelu  # swapped to Identity in sim tests (interp lacks Gelu)

N_NODES = 10000
N_EDGES = 160000
HID = 256
EDIM = 128
CAT = 2 * HID + EDIM  # 640
EPS = 1e-5
NCORES = 8
NODES_PER_CORE = (N_NODES + NCORES - 1) // NCORES  # 1250
NODES_PAD = 1280  # per-core padded node count (10 windows of 128)
NWIN = NODES_PAD // 128  # 10
TCH = 128    # edge chunk (matmul K)
TMAC = 512   # macro-tile (4 chunks)


# ---------------------------------------------------------------------------
# BIR fixup: this walrus build supports at most ONE sync wait per instruction.
# TileContext emits multi-wait instructions; split extras onto NoOps.
# ---------------------------------------------------------------------------

def _split_multiwaits(bir: dict) -> dict:
    ctr = 0
    for fn in bir["functions"]:
        for blk in fn["blocks"]:
            new_insts = []
            for inst in blk["instructions"]:
                si = inst.get("sync_info")
                ow = (si or {}).get("on_wait") or []
                if len(ow) > 1:
                    for w in ow[:-1]:
                        ctr += 1
                        new_insts.append({
                            "debug": inst.get("debug", 0),
                            "engine": inst["engine"],
                            "ins": [],
                            "outs": [],
                            "name": f"mwfix-{ctr}",
                            "opcode": "NoOp",
                            "sync_info": {"on_update": [], "on_wait": [w]},
                        })
                    si["on_wait"] = [ow[-1]]
                new_insts.append(inst)
            blk["instructions"] = new_insts
    return bir


def _patch_to_json(nc):
    orig = nc.to_json_bytes

    def patched():
        return orjson.dumps(_split_multiwaits(orjson.loads(orig())))

    nc.to_json_bytes = patched


# ---------------------------------------------------------------------------
# Device program
# ---------------------------------------------------------------------------

def build_nc(emax: int, jwin: int, win_chunk_start: list[int]):
    """Build the per-core Bass program (identical for all 8 cores).

    emax: padded per-core edge count (multiple of TMAC)
    jwin: chunks-per-window in the static aggregation schedule
    win_chunk_start: for each window w, first m-chunk index of its schedule
    """
    nmac = emax // TMAC
    nch = emax // TCH
    npairs = NWIN * jwin

    nc = bass.Bass("TRN2", target_bir_lowering=False, debug=False)

    # --- DRAM I/O ---------------------------------------------------------
    h_rep = nc.dram_tensor("h_rep", [N_NODES, HID], F32, kind="ExternalInput")
    h_node = nc.dram_tensor("h_node", [NODES_PAD, HID], F32, kind="ExternalInput")
    eT_in = nc.dram_tensor("eT_in", [EDIM, emax], F32, kind="ExternalInput")
    e_tok = nc.dram_tensor("e_tok", [emax, EDIM], F32, kind="ExternalInput")
    src_ix = nc.dram_tensor("src_ix", [emax], I32, kind="ExternalInput")
    dst_ix = nc.dram_tensor("dst_ix", [emax], I32, kind="ExternalInput")
    aggA = nc.dram_tensor("aggA", [npairs, TCH, 128], F32, kind="ExternalInput")
    inv_cnt = nc.dram_tensor("inv_cnt", [NODES_PAD], F32, kind="ExternalInput")
    We1 = nc.dram_tensor("We1", [CAT, EDIM], F32, kind="ExternalInput")
    We2 = nc.dram_tensor("We2", [EDIM, EDIM], F32, kind="ExternalInput")
    Wv1 = nc.dram_tensor("Wv1", [CAT, HID], F32, kind="ExternalInput")
    Wv2 = nc.dram_tensor("Wv2", [HID, HID], F32, kind="ExternalInput")
    be1 = nc.dram_tensor("be1", [EDIM, 1], F32, kind="ExternalInput")
    be2 = nc.dram_tensor("be2", [EDIM, 1], F32, kind="ExternalInput")
    bv1 = nc.dram_tensor("bv1", [HID, 1], F32, kind="ExternalInput")
    bv2_rep = nc.dram_tensor("bv2_rep", [128, HID], F32, kind="ExternalInput")
    ne_g4 = nc.dram_tensor("ne_g4", [128, TMAC], F32, kind="ExternalInput")
    ne_b4 = nc.dram_tensor("ne_b4", [128, TMAC], F32, kind="ExternalInput")
    nv_g_rep = nc.dram_tensor("nv_g_rep", [128, HID], F32, kind="ExternalInput")
    nv_b_rep = nc.dram_tensor("nv_b_rep", [128, HID], F32, kind="ExternalInput")
    ident = nc.dram_tensor("ident", [128, 128], F32, kind="ExternalInput")

    e_new_out = nc.dram_tensor("e_new", [emax, EDIM], F32, kind="ExternalOutput")
    h_new_out = nc.dram_tensor("h_new", [NODES_PAD, HID], F32, kind="ExternalOutput")

    KE = CAT // 128  # 5 K-chunks for MLP1
    r32 = lambda ap: ap.bitcast(F32R)

    from contextlib import ExitStack

    with tile.TileContext(nc) as tc, ExitStack() as ctx:
        ec = ctx.enter_context
        cpool = ec(tc.tile_pool(name="const", bufs=1))
        p_et = ec(tc.tile_pool(name="etin", bufs=3))
        p_etok = ec(tc.tile_pool(name="etok", bufs=3))
        p_gs = ec(tc.tile_pool(name="gsrc", bufs=2))
        p_gd = ec(tc.tile_pool(name="gdst", bufs=2))
        p_hts = ec(tc.tile_pool(name="hts", bufs=2))
        p_htd = ec(tc.tile_pool(name="htd", bufs=2))
        p_ge = ec(tc.tile_pool(name="ge", bufs=2))
        p_gv = ec(tc.tile_pool(name="gv", bufs=2))
        p_emsgT = ec(tc.tile_pool(name="emsgT", bufs=2))
        p_echain = ec(tc.tile_pool(name="echain", bufs=2))
        p_enew = ec(tc.tile_pool(name="enew", bufs=3))
        p_enewT = ec(tc.tile_pool(name="enewT", bufs=2))
        p_m = ec(tc.tile_pool(name="mpool", bufs=44))
        p_a = ec(tc.tile_pool(name="apool", bufs=4))
        p_st = ec(tc.tile_pool(name="stats", bufs=4))
        p_hn = ec(tc.tile_pool(name="hnode", bufs=2))
        p_hnew = ec(tc.tile_pool(name="hnew", bufs=2))
        # PSUM pools (8 banks total)
        ps_ht = ec(tc.tile_pool(name="ps_ht", bufs=1, space="PSUM"))   # 1
        ps_a = ec(tc.tile_pool(name="ps_a", bufs=2, space="PSUM"))     # 2
        ps_v1 = ec(tc.tile_pool(name="ps_v1", bufs=1, space="PSUM"))   # 2
        ps_v2 = ec(tc.tile_pool(name="ps_v2", bufs=1, space="PSUM"))   # 2
        ps_agg = ec(tc.tile_pool(name="ps_agg", bufs=1, space="PSUM"))  # 1
        if True:
            # --- constants -------------------------------------------------
            t_We1 = cpool.tile([128, KE, EDIM], F32R, tag="We1")
            nc.sync.dma_start(t_We1[:], We1.ap().rearrange("(k p) f -> p k f", p=128).bitcast(F32R))
            t_We2 = cpool.tile([128, EDIM], F32R, tag="We2")
            nc.sync.dma_start(t_We2[:], We2[:, :].bitcast(F32R))
            t_Wv1 = cpool.tile([128, KE, HID], F32R, tag="Wv1")
            nc.sync.dma_start(t_Wv1[:], Wv1.ap().rearrange("(k p) f -> p k f", p=128).bitcast(F32R))
            t_Wv2 = cpool.tile([128, 2, HID], F32R, tag="Wv2")
            nc.sync.dma_start(t_Wv2[:], Wv2.ap().rearrange("(k p) f -> p k f", p=128).bitcast(F32R))
            t_be1 = cpool.tile([128, 1], F32, tag="be1")
            nc.sync.dma_start(t_be1[:], be1[:, :])
            t_be2 = cpool.tile([128, 1], F32, tag="be2")
            nc.sync.dma_start(t_be2[:], be2[:, :])
            t_bv1 = cpool.tile([128, 2], F32, tag="bv1")
            nc.sync.dma_start(
                t_bv1[:], bv1.ap().rearrange("(c p) one -> p (c one)", p=128))
            t_bv2 = cpool.tile([128, HID], F32, tag="bv2")
            nc.sync.dma_start(t_bv2[:], bv2_rep[:, :])
            t_neg4 = cpool.tile([128, TMAC], F32, tag="neg4")
            nc.sync.dma_start(t_neg4[:], ne_g4[:, :])
            t_neb4 = cpool.tile([128, TMAC], F32, tag="neb4")
            nc.sync.dma_start(t_neb4[:], ne_b4[:, :])
            t_nvg = cpool.tile([128, HID], F32, tag="nvg")
            nc.sync.dma_start(t_nvg[:], nv_g_rep[:, :])
            t_nvb = cpool.tile([128, HID], F32, tag="nvb")
            nc.sync.dma_start(t_nvb[:], nv_b_rep[:, :])
            t_id = cpool.tile([128, 128], F32R, tag="ident")
            nc.sync.dma_start(t_id[:], ident[:, :].bitcast(F32R))
            t_eps = cpool.tile([128, 1], F32, tag="eps")
            nc.vector.memset(t_eps[:], EPS)
            t_icnt = cpool.tile([128, NWIN], F32, tag="icnt")
            nc.sync.dma_start(
                t_icnt[:], inv_cnt.ap().rearrange("(w p) -> p w", p=128))
            t_six = cpool.tile([128, nch], I32, tag="six")
            nc.sync.dma_start(
                t_six[:], src_ix.ap().rearrange("(c p) -> p c", p=128))
            t_dix = cpool.tile([128, nch], I32, tag="dix")
            nc.sync.dma_start(
                t_dix[:], dst_ix.ap().rearrange("(c p) -> p c", p=128))

            m_tiles = {}
            state = {"agg_done": 0}

            def emit_window(w):
                """Aggregate window w (nodes 128w..128w+127), then node LN."""
                t_agg = ps_agg.tile([128, HID], F32, tag="agg", name=f"agg{w}")
                for j in range(jwin):
                    c = win_chunk_start[w] + j
                    p = w * jwin + j
                    t_A = p_a.tile([128, 128], F32R, tag="A", name=f"A{p}")
                    nc.sync.dma_start(t_A[:], aggA[p, :, :].bitcast(F32R))
                    nc.tensor.matmul(
                        t_agg[:], r32(t_A[:]), r32(m_tiles[c][:]),
                        start=(j == 0), stop=(j == jwin - 1),
                    )
                # mean + residual: (agg * inv_cnt) + h   (one DVE op)
                t_hn = p_hn.tile([128, HID], F32, tag="hn", name=f"hn{w}")
                nc.sync.dma_start(t_hn[:], h_node[w * 128:(w + 1) * 128, :])
                t_res = p_hnew.tile([128, HID], F32, tag="hres", name=f"hres{w}")
                nc.vector.scalar_tensor_tensor(
                    t_res[:], t_agg[:], t_icnt[:, w:w + 1], t_hn[:],
                    op0=ALU.mult, op1=ALU.add,
                )
                # LayerNorm over 256 features
                t_sum = p_st.tile([128, 1], F32, tag="vsum", name=f"vsum{w}")
                nc.vector.tensor_reduce(t_sum[:], t_res[:], axis=AX.X, op=ALU.add,
                                        negate=True)
                t_nmu = p_st.tile([128, 1], F32, tag="vnmu", name=f"vnmu{w}")
                nc.vector.tensor_scalar_mul(t_nmu[:], t_sum[:], 1.0 / HID)
                t_xc = p_hnew.tile([128, HID], F32, tag="hxc", name=f"hxc{w}")
                nc.vector.tensor_scalar_add(t_xc[:], t_res[:], t_nmu[:])
                t_sq = p_hnew.tile([128, HID], F32, tag="hsq", name=f"hsq{w}")
                t_ssq = p_st.tile([128, 1], F32, tag="vssq", name=f"vssq{w}")
                nc.scalar.activation(t_sq[:], t_xc[:], AF.Square,
                                     bias=0.0, scale=1.0, accum_out=t_ssq[:])
                t_std = p_st.tile([128, 1], F32, tag="vstd", name=f"vstd{w}")
                nc.scalar.activation(t_std[:], t_ssq[:], AF.Sqrt,
                                     bias=t_eps[:], scale=1.0 / HID)
                t_rstd = p_st.tile([128, 1], F32, tag="vrstd", name=f"vrstd{w}")
                nc.vector.reciprocal(t_rstd[:], t_std[:])
                t_xn = p_hnew.tile([128, HID], F32, tag="hxn", name=f"hxn{w}")
                nc.vector.tensor_scalar_mul(t_xn[:], t_xc[:], t_rstd[:])
                t_o1 = p_hnew.tile([128, HID], F32, tag="ho1", name=f"ho1{w}")
                nc.vector.tensor_tensor(t_o1[:], t_xn[:], t_nvg[:], ALU.mult)
                t_o2 = p_hnew.tile([128, HID], F32, tag="ho2", name=f"ho2{w}")
                nc.vector.tensor_tensor(t_o2[:], t_o1[:], t_nvb[:], ALU.add)
                nc.sync.dma_start(h_new_out[w * 128:(w + 1) * 128, :], t_o2[:])

            for mi in range(nmac):
                c0 = mi * 4
                sfx = f"_{mi}"
                # --- load e feature-major + token-major -----------------------
                t_eT = p_et.tile([128, TMAC], F32R, tag="eT", name=f"eT{sfx}")
                nc.sync.dma_start(t_eT[:], eT_in[:, mi * TMAC:(mi + 1) * TMAC].bitcast(F32R))
                t_etok = p_etok.tile([128, 4, EDIM], F32, tag="etok",
                                     name=f"etok{sfx}")
                nc.sync.dma_start(
                    t_etok[:],
                    e_tok.ap()[mi * TMAC:(mi + 1) * TMAC, :]
                    .rearrange("(c p) f -> p c f", p=128),
                )
                # --- gather h_src / h_dst (token-major) -----------------------
                t_gs = p_gs.tile([128, 4, HID], F32R, tag="gs", name=f"gs{sfx}")
                t_gd = p_gd.tile([128, 4, HID], F32R, tag="gd", name=f"gd{sfx}")
                for c in range(4):
                    nc.gpsimd.indirect_dma_start(
                        t_gs[:, c, :], None,
                        h_rep.ap().bitcast(F32R),
                        IndirectOffsetOnAxis(ap=t_six[:, c0 + c:c0 + c + 1], axis=0),
                    )
                    nc.gpsimd.indirect_dma_start(
                        t_gd[:, c, :], None,
                        h_rep.ap().bitcast(F32R),
                        IndirectOffsetOnAxis(ap=t_dix[:, c0 + c:c0 + c + 1], axis=0),
                    )
                # --- transpose gathers to feature-major -----------------------
                # hT layout: [128, 2(fchunk), 4(chunk), 128(edge)]
                t_hts = p_hts.tile([128, 2, 4, 128], F32R, tag="hts",
                                   name=f"hts{sfx}")
                t_htd = p_htd.tile([128, 2, 4, 128], F32R, tag="htd",
                                   name=f"htd{sfx}")
                for half in range(2):
                    t_ps = ps_ht.tile([128, 512], F32, tag="ph",
                                      name=f"ps{sfx}_{half}")
                    for c in range(4):
                        nc.tensor.transpose(
                            r32(t_ps[:, c * 128:(c + 1) * 128]),
                            r32(t_gs[:, c, half * 128:(half + 1) * 128]),
                            r32(t_id[:]),
                        )
                    nc.vector.tensor_copy(
                        t_hts[:, half].rearrange("p c f -> p (c f)"), t_ps[:])
                    t_pd = ps_ht.tile([128, 512], F32, tag="ph",
                                      name=f"pd{sfx}_{half}")
                    for c in range(4):
                        nc.tensor.transpose(
                            r32(t_pd[:, c * 128:(c + 1) * 128]),
                            r32(t_gd[:, c, half * 128:(half + 1) * 128]),
                            r32(t_id[:]),
                        )
                    nc.vector.tensor_copy(
                        t_htd[:, half].rearrange("p c f -> p (c f)"), t_pd[:])

                def mlp1_rhs(k):
                    if k < 2:
                        return t_hts[:, k].rearrange("p c f -> p (c f)")
                    if k < 4:
                        return t_htd[:, k - 2].rearrange("p c f -> p (c f)")
                    return t_eT[:]

                # --- edge MLP1: pre_e1^T [128, 512] ---------------------------
                t_pe1 = ps_a.tile([128, TMAC], F32, tag="psa", name=f"pe1{sfx}")
                for k in range(KE):
                    nc.tensor.matmul(
                        t_pe1[:], r32(t_We1[:, k, :]), r32(mlp1_rhs(k)),
                        start=(k == 0), stop=(k == KE - 1),
                    )
                t_ge = p_ge.tile([128, TMAC], F32R, tag="ge", name=f"ge{sfx}")
                nc.scalar.activation(t_ge[:], t_pe1[:], GELU, bias=t_be1[:])

                # --- edge MLP2 (weights-stationary): e_msg^T = We2^T @ ge -----
                t_pe2 = ps_a.tile([128, TMAC], F32, tag="psa", name=f"pe2{sfx}")
                nc.tensor.matmul(t_pe2[:], r32(t_We2[:]), r32(t_ge[:]),
                                 start=True, stop=True)
                t_emT = p_emsgT.tile([128, TMAC], F32R, tag="emT", name=f"emT{sfx}")
                nc.vector.tensor_scalar_add(t_emT[:], t_pe2[:], t_be2[:])

                # --- transpose e_msg to token-major ---------------------------
                t_pet = ps_a.tile([128, TMAC], F32, tag="psa", name=f"pet{sfx}")
                for c in range(4):
                    nc.tensor.transpose(
                        r32(t_pet[:, c * 128:(c + 1) * 128]),
                        r32(t_emT[:, c * 128:(c + 1) * 128]),
                        r32(t_id[:]),
                    )
                # residual: x = e_msg + e  (token-major [128, 4, 128])
                t_ex = p_echain.tile([128, 4, EDIM], F32, tag="ex", name=f"ex{sfx}")
                nc.vector.tensor_tensor(
                    t_ex.rearrange("p c f -> p (c f)"), t_pet[:],
                    t_etok.rearrange("p c f -> p (c f)"), ALU.add)
                # --- edge LayerNorm (per token over 128 features) -------------
                t_es = p_st.tile([128, 4], F32, tag="esum", name=f"es{sfx}")
                nc.vector.tensor_reduce(t_es[:], t_ex[:], axis=AX.X, op=ALU.add,
                                        negate=True)
                t_enmu = p_st.tile([128, 4], F32, tag="enmu", name=f"enmu{sfx}")
                nc.vector.tensor_scalar_mul(t_enmu[:], t_es[:], 1.0 / EDIM)
                t_exc = p_echain.tile([128, 4, EDIM], F32, tag="exc",
                                      name=f"exc{sfx}")
                nc.vector.tensor_tensor(
                    t_exc[:], t_ex[:],
                    t_enmu.rearrange("p (c one) -> p c one", one=1)
                    .broadcast_to([128, 4, EDIM]),
                    ALU.add)
                t_esq = p_echain.tile([128, 4, EDIM], F32, tag="esq",
                                      name=f"esq{sfx}")
                nc.vector.tensor_tensor(
                    t_esq.rearrange("p c f -> p (c f)"),
                    t_exc.rearrange("p c f -> p (c f)"),
                    t_exc.rearrange("p c f -> p (c f)"), ALU.mult)
                t_essq = p_st.tile([128, 4], F32, tag="essq", name=f"essq{sfx}")
                nc.vector.tensor_reduce(t_essq[:], t_esq[:], axis=AX.X, op=ALU.add)
                t_estd = p_st.tile([128, 4], F32, tag="estd", name=f"estd{sfx}")
                nc.scalar.activation(t_estd[:], t_essq[:], AF.Sqrt,
                                     bias=t_eps[:], scale=1.0 / EDIM)
                t_erstd = p_st.tile([128, 4], F32, tag="erstd", name=f"erstd{sfx}")
                nc.vector.reciprocal(t_erstd[:], t_estd[:])
                t_exn = p_echain.tile([128, 4, EDIM], F32, tag="exn",
                                      name=f"exn{sfx}")
                nc.vector.tensor_tensor(
                    t_exn[:], t_exc[:],
                    t_erstd.rearrange("p (c one) -> p c one", one=1)
                    .broadcast_to([128, 4, EDIM]),
                    ALU.mult)
                t_eg = p_enew.tile([128, 4, EDIM], F32, tag="eg", name=f"eg{sfx}")
                nc.vector.tensor_tensor(
                    t_eg.rearrange("p c f -> p (c f)"),
                    t_exn.rearrange("p c f -> p (c f)"),
                    t_neg4[:], ALU.mult)
                t_enew = p_enew.tile([128, 4, EDIM], F32R, tag="enew",
                                     name=f"enew{sfx}")
                nc.vector.tensor_tensor(
                    t_enew.rearrange("p c f -> p (c f)"),
                    t_eg.rearrange("p c f -> p (c f)"),
                    t_neb4[:], ALU.add)
                nc.sync.dma_start(
                    e_new_out.ap()[mi * TMAC:(mi + 1) * TMAC, :]
                    .rearrange("(c p) f -> p c f", p=128).bitcast(F32R),
                    t_enew[:])
                # --- transpose e_new to feature-major -------------------------
                t_pent = ps_a.tile([128, TMAC], F32, tag="psa", name=f"pent{sfx}")
                for c in range(4):
                    nc.tensor.transpose(
                        r32(t_pent[:, c * 128:(c + 1) * 128]),
                        r32(t_enew[:, c, :]),
                        r32(t_id[:]),
                    )
                t_enT = p_enewT.tile([128, TMAC], F32R, tag="enT", name=f"enT{sfx}")
                nc.vector.tensor_copy(t_enT[:], t_pent[:])

                # --- node MLP1: pre_v1^T  -------------------------------------
                t_gv = p_gv.tile([128, 2, TMAC], F32R, tag="gv", name=f"gv{sfx}")
                for h2 in range(2):
                    t_pv1 = ps_v1.tile([128, TMAC], F32, tag=f"psv1{h2}",
                                       name=f"pv1{sfx}_{h2}")
                    for k in range(KE):
                        rhs = t_enT[:] if k == 4 else mlp1_rhs(k)
                        nc.tensor.matmul(
                            t_pv1[:], r32(t_Wv1[:, k, h2 * 128:(h2 + 1) * 128]),
                            r32(rhs), start=(k == 0), stop=(k == KE - 1),
                        )
                    nc.scalar.activation(t_gv[:, h2, :], t_pv1[:], GELU,
                                         bias=t_bv1[:, h2:h2 + 1])

                # --- node MLP2 (activation-stationary, token-major) -----------
                for c in range(4):
                    t_pv2 = ps_v2.tile([128, HID], F32, tag=f"psv2{c % 2}",
                                       name=f"pv2{sfx}_{c}")
                    for k2 in range(2):
                        nc.tensor.matmul(
                            t_pv2[:],
                            r32(t_gv[:, k2, c * 128:(c + 1) * 128]),
                            r32(t_Wv2[:, k2, :]),
                            start=(k2 == 0), stop=(k2 == 1),
                        )
                    t_m = p_m.tile([128, HID], F32R, tag="m", name=f"m_{c0 + c}")
                    nc.vector.tensor_tensor(t_m[:], t_pv2[:], t_bv2[:], ALU.add)
                    m_tiles[c0 + c] = t_m

                # --- emit windows whose chunks are all produced ---------------
                while state["agg_done"] < NWIN:
                    w = state["agg_done"]
                    if win_chunk_start[w] + jwin - 1 <= c0 + 3:
                        emit_window(w)
                        state["agg_done"] += 1
                    else:
                        break

            while state["agg_done"] < NWIN:
                emit_window(state["agg_done"])
                state["agg_done"] += 1

    _patch_to_json(nc)
    return nc


# ---------------------------------------------------------------------------
# Host side: preprocessing, run, postprocessing
# ---------------------------------------------------------------------------

def _preprocess(h, e, edge_index):
    src = np.asarray(edge_index[0], dtype=np.int64)
    dst = np.asarray(edge_index[1], dtype=np.int64)
    perm = np.argsort(dst, kind="stable")
    src_s, dst_s = src[perm], dst[perm]
    e_s = np.asarray(e, dtype=np.float32)[perm]

    bounds = np.arange(NCORES + 1) * NODES_PER_CORE
    bounds[-1] = max(bounds[-1], N_NODES)
    starts = np.searchsorted(dst_s, bounds)
    ecnt = np.diff(starts)
    emax = int(math.ceil(max(1, int(ecnt.max())) / TMAC) * TMAC)

    counts = np.bincount(dst, minlength=N_NODES).astype(np.float32)
    invc_full = (1.0 / np.maximum(counts, 1.0)).astype(np.float32)

    cores = []
    for k in range(NCORES):
        s0, s1 = int(starts[k]), int(starts[k + 1])
        n = s1 - s0
        sk = np.zeros(emax, dtype=np.int32)
        dk = np.full(emax, -1, dtype=np.int64)
        ek = np.zeros((emax, EDIM), dtype=np.float32)
        sk[:n] = src_s[s0:s1]
        dk[:n] = dst_s[s0:s1]
        ek[:n] = e_s[s0:s1]
        nlo = k * NODES_PER_CORE
        nhi = min(N_NODES, (k + 1) * NODES_PER_CORE)
        hn = np.zeros((NODES_PAD, HID), dtype=np.float32)
        hn[: nhi - nlo] = h[nlo:nhi]
        ic = np.ones(NODES_PAD, dtype=np.float32)
        ic[: nhi - nlo] = invc_full[nlo:nhi]
        cores.append(dict(src=sk, dst=dk, e=ek, n=n, nlo=nlo, h_node=hn,
                          inv_cnt=ic))
    return cores, perm, emax, starts


def _agg_schedule(cores, emax, jwin_min=4, guard=2):
    """Static (window -> chunk range) schedule shared by all cores."""
    nch = emax // TCH
    need_lo = [nch] * NWIN
    need_hi = [-1] * NWIN
    for core in cores:
        dloc = core["dst"] - core["nlo"]
        dloc = np.where(core["dst"] < 0, -1, dloc)
        chunk_of = np.arange(emax) // TCH
        for w in range(NWIN):
            sel = (dloc >= w * 128) & (dloc < (w + 1) * 128)
            if sel.any():
                ch = chunk_of[sel]
                need_lo[w] = min(need_lo[w], int(ch[0]))
                need_hi[w] = max(need_hi[w], int(ch[-1]))
    jwin = jwin_min
    for w in range(NWIN):
        if need_hi[w] >= 0:
            jwin = max(jwin, need_hi[w] - need_lo[w] + 1 + guard)
    jwin = min(jwin, nch)
    wcs = []
    for w in range(NWIN):
        if need_hi[w] >= 0:
            lo = need_lo[w] - guard // 2
        else:
            lo = (w * nch) // NWIN
        lo = max(0, min(lo, nch - jwin))
        wcs.append(lo)
        if need_hi[w] >= 0:
            assert lo <= need_lo[w] and need_hi[w] <= lo + jwin - 1, (
                "aggregation schedule cannot cover window", w, lo, need_lo[w],
                need_hi[w], jwin)
    return jwin, wcs


def _build_A(core, emax, jwin, wcs):
    nch = emax // TCH
    A = np.zeros((NWIN * jwin, TCH, 128), dtype=np.float32)
    dloc = core["dst"] - core["nlo"]
    dloc = np.where(core["dst"] < 0, -1, dloc)
    for w in range(NWIN):
        for j in range(jwin):
            c = wcs[w] + j
            if not (0 <= c < nch):
                continue
            p = w * jwin + j
            dl = dloc[c * TCH:(c + 1) * TCH]
            ok = (dl >= w * 128) & (dl < (w + 1) * 128)
            r = np.nonzero(ok)[0]
            A[p, r, (dl[r] - w * 128).astype(np.int64)] = 1.0
    return A


_RUNNER_CACHE = {}


class _Runner:
    def __init__(self, emax, jwin, wcs):
        import jax
        from jax.sharding import Mesh, PartitionSpec
        from jax.experimental.shard_map import shard_map
        from concourse import bass2jax

        self.nc = build_nc(emax, jwin, wcs)
        nc = self.nc
        bass2jax.install_neuronx_cc_hook()

        part_name = (nc.partition_id_tensor.name
                     if nc.partition_id_tensor else None)
        in_names, out_names, out_avals = [], [], []
        for alloc in nc.m.functions[0].allocations:
            if not isinstance(alloc, mybir.MemoryLocationSet):
                continue
            name = alloc.memorylocations[0].name
            if alloc.kind == "ExternalInput":
                if name != part_name:
                    in_names.append(name)
            elif alloc.kind == "ExternalOutput":
                out_names.append(name)
                out_avals.append(jax.core.ShapedArray(
                    tuple(alloc.tensor_shape), mybir.dt.np(alloc.dtype)))
        self.in_names, self.out_names, self.out_avals = (
            in_names, out_names, out_avals)
        n_params = len(in_names)
        all_names = in_names + out_names
        if part_name is not None:
            all_names = all_names + [part_name]

        def _body(*args):
            operands = list(args)
            if part_name is not None:
                operands.append(bass2jax.partition_id_tensor())
            outs = bass2jax._bass_exec_p.bind(
                *operands,
                out_avals=tuple(out_avals),
                in_names=tuple(all_names),
                out_names=tuple(out_names),
                lowering_input_output_aliases=(),
                sim_require_finite=True,
                sim_require_nnan=True,
                nc=nc,
            )
            return tuple(outs)

        devices = jax.devices()[:NCORES]
        mesh = Mesh(np.asarray(devices), ("core",))
        spec = (PartitionSpec("core"),)
        self.sharded = jax.jit(shard_map(
            _body, mesh=mesh,
            in_specs=spec * (n_params + len(out_names)),
            out_specs=spec * len(out_names),
            check_rep=False,
        ))
        self.jax = jax

    def place_inputs(self, in_maps):
        jax = self.jax
        concat = [
            np.concatenate([np.asarray(m[nm]) for m in in_maps], axis=0)
            for nm in self.in_names
        ]
        zeros = [
            np.zeros((NCORES * av.shape[0], *av.shape[1:]), av.dtype)
            for av in self.out_avals
        ]
        return [jax.device_put(a) for a in concat + zeros]

    def run(self, dev_args):
        outs = self.sharded(*dev_args)
        self.jax.block_until_ready(outs)
        return outs

    def split_outputs(self, outs):
        res = []
        for c in range(NCORES):
            res.append({
                nm: np.asarray(outs[i]).reshape(
                    NCORES, *self.out_avals[i].shape)[c]
                for i, nm in enumerate(self.out_names)
            })
        return res


def _get_runner(emax, jwin, wcs):
    key = (emax, jwin, tuple(wcs))
    if key not in _RUNNER_CACHE:
        _RUNNER_CACHE[key] = _Runner(emax, jwin, wcs)
    return _RUNNER_CACHE[key]


def make_in_maps(inputs):
    """Full preprocessing: returns (runner, in_maps, assemble_fn)."""
    h = np.asarray(inputs["h"], dtype=np.float32)
    e = np.asarray(inputs["e"], dtype=np.float32)
    edge_index = np.asarray(inputs["edge_index"])
    cores, perm, emax, _starts = _preprocess(h, e, edge_index)
    jwin, wcs = _agg_schedule(cores, emax)

    We1 = np.asarray(inputs["We1"], np.float32)
    We2 = np.asarray(inputs["We2"], np.float32)
    Wv1 = np.asarray(inputs["Wv1"], np.float32)
    Wv2 = np.asarray(inputs["Wv2"], np.float32)
    be1 = np.asarray(inputs["be1"], np.float32).reshape(EDIM, 1)
    be2 = np.asarray(inputs["be2"], np.float32).reshape(EDIM, 1)
    bv1 = np.asarray(inputs["bv1"], np.float32).reshape(HID, 1)
    bv2_rep = np.tile(np.asarray(inputs["bv2"], np.float32)[None, :], (128, 1))
    ne_g4 = np.tile(np.asarray(inputs["ne_g"], np.float32)[None, :], (128, 4))
    ne_b4 = np.tile(np.asarray(inputs["ne_b"], np.float32)[None, :], (128, 4))
    nv_g_rep = np.tile(np.asarray(inputs["nv_g"], np.float32)[None, :], (128, 1))
    nv_b_rep = np.tile(np.asarray(inputs["nv_b"], np.float32)[None, :], (128, 1))
    ident = np.eye(128, dtype=np.float32)

    in_maps = []
    for k in range(NCORES):
        c = cores[k]
        in_maps.append({
            "h_rep": h,
            "h_node": c["h_node"],
            "eT_in": np.ascontiguousarray(c["e"].T),
            "e_tok": c["e"],
            "src_ix": c["src"],
            "dst_ix": np.maximum(c["dst"], 0).astype(np.int32),
            "aggA": _build_A(c, emax, jwin, wcs),
            "inv_cnt": c["inv_cnt"],
            "We1": We1, "We2": We2, "Wv1": Wv1, "Wv2": Wv2,
            "be1": be1, "be2": be2, "bv1": bv1, "bv2_rep": bv2_rep,
            "ne_g4": ne_g4, "ne_b4": ne_b4,
            "nv_g_rep": nv_g_rep, "nv_b_rep": nv_b_rep,
            "ident": ident,
        })

    runner = _get_runner(emax, jwin, wcs)

    def assemble(results):
        h_new = np.empty((N_NODES, HID), dtype=np.float32)
        e_new = np.empty((perm.shape[0], EDIM), dtype=np.float32)
        for k in range(NCORES):
            nlo = cores[k]["nlo"]
            nhi = min(N_NODES, nlo + NODES_PER_CORE)
            h_new[nlo:nhi] = results[k]["h_new"][: nhi - nlo]
        e_sorted = np.concatenate(
            [results[k]["e_new"][: cores[k]["n"]] for k in range(NCORES)],
            axis=0)
        e_new[perm] = e_sorted
        return h_new, e_new

    return runner, in_maps, assemble


def kernel(**inputs):
    runner, in_maps, assemble = make_in_maps(inputs)
    dev_args = runner.place_inputs(in_maps)
    outs = runner.run(dev_args)
    return assemble(runner.split_outputs(outs))
